# revision 13
# baseline (speedup 1.0000x reference)
"""Trainium2 Bass kernel: batched 1-D linear interpolation on a uniform grid.

out[b, j] = (1 - w_j) * y[b, i_j] + w_j * y[b, i_j + 1]

Reformulated as a matmul  out = y @ G  with G[i_j, j] = 1 - w_j and
G[i_j + 1, j] = w_j (2 nonzeros per column, known on the host from x_new).

Device layout: y is shipped pre-transposed (points-major) in bf16.  The grid
is covered by 17 fixed windows of 128 consecutive points starting every 127
points, so a query with i_j in [127k, 127k+126] finds both of its points in
window k.  Queries are sorted by i_j and grouped into 32 blocks of 128; the
small G block is the PE's *stationary* operand and the y-window rows stream
through in 512-column pieces:

    psum[q128, rows512] += G_blk[pts128, q128].T @ win_k[pts128, rows512]

A block straddling a window boundary is computed as two accumulating
matmuls (one per window, zero columns elsewhere) — PSUM's per-element
has_written bit makes the sum exact.  One stationary load serves a long
stream, so the PE runs near its issue floor.  PSUM banks are cast to bf16
on the way to SBUF (DVE/ACT split) and the transposed output [4096 sorted
queries, 2048 rows] streams out in 512 KiB strips on the second HWDGE ring,
overlapping the window loads on the sync ring.  The host transposes and
un-permutes during the unshard.

Sharding: pure data parallel over the batch axis across 8 NeuronCores
(y_points rows 16384 -> 8 x 2048); x_new-derived constants are replicated.
"""

import numpy as np

BATCH = 16384
NUM_POINTS = 2048
M = 4096
N_CORES = 8
ROWS_PER_CORE = BATCH // N_CORES  # 2048
P = 128
WGRID = P - 1  # window stride: 127 points
NWIN = (NUM_POINTS - 2) // WGRID + 1  # 17
QB = M // P  # 32 query blocks of 128 sorted queries
RB = 512  # rows streamed per matmul (one PSUM bank of fp32)
NRB = ROWS_PER_CORE // RB  # 4

_NC_CACHE = {}
_PLAN_CACHE = {}


def _host_precompute(x_new):
    """Replicate the reference's searchsorted/weight math with the same jax
    ops on the same backend, so boundary decisions match the reference."""
    import jax.numpy as jnp

    x_new_j = jnp.asarray(np.asarray(x_new, dtype=np.float32))
    x_points = jnp.linspace(0.0, 1.0, NUM_POINTS, dtype=x_new_j.dtype)
    idxs = jnp.searchsorted(x_points, x_new_j, side="right") - 1
    idxs = jnp.clip(idxs, 0, NUM_POINTS - 2)
    x1 = x_points[idxs]
    x2 = x_points[idxs + 1]
    w = (x_new_j - x1) / (x2 - x1)
    return np.asarray(idxs).astype(np.int64), np.asarray(w, dtype=np.float32)


def _win_p0(k):
    return min(WGRID * k, NUM_POINTS - P)


def _make_plan(x_new):
    """Sort queries by bin index; per 128-query block, list the windows its
    queries fall in.  Returns (pairs, order, idxs, w): pairs[qb] = [k, ...]
    ascending (usually 1-2 windows)."""
    idxs, w = _host_precompute(x_new)
    order = np.argsort(idxs, kind="stable")
    si = idxs[order]
    kq = si // WGRID  # window id per sorted query

    pairs = []
    for qb in range(QB):
        ks = np.unique(kq[qb * P : (qb + 1) * P])
        pairs.append([int(k) for k in ks])
    return pairs, order, idxs, w


def _build_nc(pairs, n_stat):
    import concourse.bacc as bacc
    import concourse.mybir as mybir
    from concourse.tile import TileContext

    f32 = mybir.dt.float32
    bf16 = mybir.dt.bfloat16

    nc = bacc.Bacc()
    yt = nc.dram_tensor("yt", [NUM_POINTS, ROWS_PER_CORE], bf16, kind="ExternalInput")
    g = nc.dram_tensor("g", [P, n_stat * P], bf16, kind="ExternalInput")
    out = nc.dram_tensor("out", [M, ROWS_PER_CORE], bf16, kind="ExternalOutput")

    with TileContext(nc) as tc:
        with (
            tc.tile_pool(name="const", bufs=1) as cp,
            tc.tile_pool(name="psum", bufs=8, space="PSUM") as pp,
            tc.tile_pool(name="outp", bufs=6) as op,
        ):
            g_t = cp.tile([P, n_stat * P], bf16, tag="g")
            win = [
                cp.tile([P, ROWS_PER_CORE], bf16, tag=f"win{k}", name=f"win{k}")
                for k in range(NWIN)
            ]
            # issue order tuned for pipeline fill: the first block's matmuls
            # need win0 rows 0:512 and only the first couple of g groups
            p0 = _win_p0(0)
            nc.sync.dma_start(out=g_t[:, : 4 * P], in_=g[:, : 4 * P])
            nc.sync.dma_start(out=win[0][:, :RB], in_=yt[p0 : p0 + P, :RB])
            nc.sync.dma_start(out=win[0][:, RB:], in_=yt[p0 : p0 + P, RB:])
            gq = (n_stat * P - 4 * P) // 3
            for s in range(3):
                lo = 4 * P + s * gq
                hi = 4 * P + (s + 1) * gq if s < 2 else n_stat * P
                nc.sync.dma_start(out=g_t[:, lo:hi], in_=g[:, lo:hi])
            for k in range(1, NWIN):
                p0 = _win_p0(k)
                nc.sync.dma_start(out=win[k][:], in_=yt[p0 : p0 + P, :])

            stat = 0
            cnt = 0
            for qb in range(QB):
                ks = pairs[qb]
                ot = op.tile([P, ROWS_PER_CORE], bf16, tag="ot", name="ot")
                for rb in range(NRB):
                    ps = pp.tile([P, RB], f32, tag="ps")
                    for t, k in enumerate(ks):
                        nc.tensor.matmul(
                            ps[:],
                            g_t[:, (stat + t) * P : (stat + t + 1) * P],
                            win[k][:, rb * RB : (rb + 1) * RB],
                            start=(t == 0),
                            stop=(t == len(ks) - 1),
                        )
                    dst = ot[:, rb * RB : (rb + 1) * RB]
                    # alternate the PSUM->SBUF cast copies between DVE and
                    # ACT copy-by-copy (runs on one engine would serialize
                    # PSUM bank recycling and stall the matmul stream);
                    # DVE takes the odd extra — ACT also issues out-DMAs
                    if cnt % 13 % 2 == 0:
                        nc.vector.tensor_copy(out=dst, in_=ps[:])
                    else:
                        nc.scalar.copy(dst, ps[:])
                    cnt += 1
                stat += len(ks)
                # transposed output strip [128 sorted queries, all rows].
                # Early strips ride the second HWDGE ring (ACT) to overlap
                # the window loads on the sync ring; late strips move to the
                # sync ring (drained by then) to offload the ACT sequencer.
                eng = nc.scalar if qb < QB // 2 else nc.sync
                eng.dma_start(out=out[qb * P : (qb + 1) * P, :], in_=ot[:])

    nc.compile()
    return nc


def _get_plan_and_nc(x_new):
    import ml_dtypes

    key = np.asarray(x_new, dtype=np.float32).tobytes()
    if key not in _PLAN_CACHE:
        pairs, order, idxs, w = _make_plan(x_new)
        si = idxs[order]
        sw = w[order]
        kq = si // WGRID
        n_stat = sum(len(ks) for ks in pairs)
        # G stationaries: [128 pts-in-window, n_stat*128], one 128-col group
        # per (block, window) pair; zero cols for queries of other windows
        gmat = np.zeros((P, n_stat * P), dtype=np.float32)
        stat = 0
        for qb, ks in enumerate(pairs):
            sl = slice(qb * P, (qb + 1) * P)
            for k in ks:
                mask = kq[sl] == k
                cols = stat * P + np.flatnonzero(mask)
                siq = si[sl][mask]
                swq = sw[sl][mask]
                p0 = _win_p0(k)
                gmat[siq - p0, cols] = 1.0 - swq
                gmat[siq + 1 - p0, cols] = swq
                stat += 1
        gmat = gmat.astype(ml_dtypes.bfloat16)
        inv = np.argsort(order, kind="stable")  # natural col -> sorted pos
        _PLAN_CACHE[key] = (pairs, n_stat, gmat, inv)
    pairs, n_stat, gmat, inv = _PLAN_CACHE[key]

    nc_key = tuple(tuple(ks) for ks in pairs)
    if nc_key not in _NC_CACHE:
        _NC_CACHE[nc_key] = _build_nc(pairs, n_stat)
    return gmat, inv, _NC_CACHE[nc_key]


def run(y_points, x_new, trace=False, **spmd_kwargs):
    """Run the Bass kernel; returns (output, BassKernelResults)."""
    import ml_dtypes
    from concourse.bass_utils import run_bass_kernel_spmd

    gmat, inv, nc = _get_plan_and_nc(x_new)

    y16 = np.asarray(y_points, dtype=np.float32).astype(ml_dtypes.bfloat16)
    in_maps = []
    for c in range(N_CORES):
        ytc = np.ascontiguousarray(y16[c * ROWS_PER_CORE : (c + 1) * ROWS_PER_CORE].T)
        in_maps.append({"yt": ytc, "g": gmat})

    res = run_bass_kernel_spmd(
        nc, in_maps, list(range(N_CORES)), trace=trace, **spmd_kwargs
    )

    out_full = np.empty((BATCH, M), dtype=np.float32)
    for c in range(N_CORES):
        o = res.results[c]["out"]  # [M sorted queries, rows] bf16
        nat = np.asarray(o).astype(np.float32)[inv, :]  # row-gather to natural
        out_full[c * ROWS_PER_CORE : (c + 1) * ROWS_PER_CORE, :] = nat.T
    return out_full, res


def kernel(y_points, x_new):
    out, _ = run(y_points, x_new)
    return out


# revision 18
# speedup vs baseline: 1.1597x; 1.1597x over previous
"""Trainium2 Bass kernel: batched 1-D linear interpolation on a uniform grid.

out[b, j] = (1 - w_j) * y[b, i_j] + w_j * y[b, i_j + 1]

Reformulated as a matmul  out = y @ G  with G[i_j, j] = 1 - w_j and
G[i_j + 1, j] = w_j (2 nonzeros per column, known on the host from x_new).

Device layout: y is shipped pre-transposed (points-major) in bf16.  The grid
is covered by 17 fixed windows of 128 consecutive points starting every 127
points, so a query with i_j in [127k, 127k+126] finds both of its points in
window k.  Queries are sorted by i_j and grouped into 32 blocks of 128; the
small G block is the PE's *stationary* operand and the y-window rows stream
through in 512-column pieces:

    psum[q128, rows512] += G_blk[pts128, q128].T @ win_k[pts128, rows512]

A block straddling a window boundary is computed as two accumulating
matmuls (one per window, zero columns elsewhere) — PSUM's per-element
has_written bit makes the sum exact.  One stationary load serves a long
stream, so the PE runs near its issue floor.  PSUM banks are cast to bf16
on the way to SBUF (DVE/ACT split) and the transposed output [4096 sorted
queries, 2048 rows] streams out in 512 KiB strips on the second HWDGE ring,
overlapping the window loads on the sync ring.  The host transposes and
un-permutes during the unshard.

Sharding: pure data parallel over the batch axis across 8 NeuronCores
(y_points rows 16384 -> 8 x 2048); x_new-derived constants are replicated.
"""

import numpy as np

BATCH = 16384
NUM_POINTS = 2048
M = 4096
N_CORES = 8
ROWS_PER_CORE = BATCH // N_CORES  # 2048
P = 128
WGRID = P - 1  # window stride: 127 points
NWIN = (NUM_POINTS - 2) // WGRID + 1  # 17
QB = M // P  # 32 query blocks of 128 sorted queries
RB = 512  # rows streamed per matmul (one PSUM bank of fp32)
NRB = ROWS_PER_CORE // RB  # 4

_NC_CACHE = {}
_PLAN_CACHE = {}


def _host_precompute(x_new):
    """Replicate the reference's searchsorted/weight math with the same jax
    ops on the same backend, so boundary decisions match the reference."""
    import jax.numpy as jnp

    x_new_j = jnp.asarray(np.asarray(x_new, dtype=np.float32))
    x_points = jnp.linspace(0.0, 1.0, NUM_POINTS, dtype=x_new_j.dtype)
    idxs = jnp.searchsorted(x_points, x_new_j, side="right") - 1
    idxs = jnp.clip(idxs, 0, NUM_POINTS - 2)
    x1 = x_points[idxs]
    x2 = x_points[idxs + 1]
    w = (x_new_j - x1) / (x2 - x1)
    return np.asarray(idxs).astype(np.int64), np.asarray(w, dtype=np.float32)


TAIL_K = NWIN - 1  # last window: only 16 unique points [2032, 2047]
TAIL_PTS = NUM_POINTS - WGRID * TAIL_K  # 16
TAIL_BASE = 64  # partitions [64, 80) hold them (auto tile_position caps at 64)


def _win_p0(k):
    """DRAM point row of the first loaded partition of window k."""
    return WGRID * k


def _win_row(k, i):
    """Tile partition row of point i within window k."""
    return i - WGRID * k + (TAIL_BASE if k == TAIL_K else 0)


def _make_plan(x_new):
    """Sort queries by bin index; per 128-query block, list the windows its
    queries fall in.  Returns (pairs, order, idxs, w): pairs[qb] = [k, ...]
    ascending (usually 1-2 windows)."""
    idxs, w = _host_precompute(x_new)
    order = np.argsort(idxs, kind="stable")
    si = idxs[order]
    kq = si // WGRID  # window id per sorted query

    pairs = []
    for qb in range(QB):
        ks = np.unique(kq[qb * P : (qb + 1) * P])
        pairs.append([int(k) for k in ks])
    return pairs, order, idxs, w


def _build_nc(pairs, n_stat):
    import concourse.bacc as bacc
    import concourse.mybir as mybir
    from concourse.tile import TileContext

    f32 = mybir.dt.float32
    bf16 = mybir.dt.bfloat16

    nc = bacc.Bacc()
    yt = nc.dram_tensor("yt", [NUM_POINTS, ROWS_PER_CORE], bf16, kind="ExternalInput")
    g = nc.dram_tensor("g", [P, n_stat * P], bf16, kind="ExternalInput")
    out = nc.dram_tensor("out", [M, ROWS_PER_CORE], bf16, kind="ExternalOutput")

    with TileContext(nc) as tc:
        with (
            tc.tile_pool(name="const", bufs=1) as cp,
            tc.tile_pool(name="psum", bufs=8, space="PSUM") as pp,
            tc.tile_pool(name="outp", bufs=6) as op,
        ):
            g_t = cp.tile([P, n_stat * P], bf16, tag="g")
            win = [
                cp.tile([P, ROWS_PER_CORE], bf16, tag=f"win{k}", name=f"win{k}")
                for k in range(NWIN)
            ]
            # issue order tuned for pipeline fill: the first block's matmuls
            # need win0 rows 0:512 and only the first couple of g groups
            p0 = _win_p0(0)
            nc.sync.dma_start(out=g_t[:, : 4 * P], in_=g[:, : 4 * P])
            nc.sync.dma_start(out=win[0][:, :RB], in_=yt[p0 : p0 + P, :RB])
            nc.sync.dma_start(out=win[0][:, RB:], in_=yt[p0 : p0 + P, RB:])
            gq = (n_stat * P - 4 * P) // 3
            for s in range(3):
                lo = 4 * P + s * gq
                hi = 4 * P + (s + 1) * gq if s < 2 else n_stat * P
                nc.sync.dma_start(out=g_t[:, lo:hi], in_=g[:, lo:hi])
            for k in range(1, NWIN - 1):
                p0 = _win_p0(k)
                nc.sync.dma_start(out=win[k][:], in_=yt[p0 : p0 + P, :])
            # last window: only the 16 points [2032, 2047] are not already
            # in window 15 — load just those (partial-partition DMA, but
            # tiny and off the critical path)
            nc.sync.dma_start(
                out=win[TAIL_K][TAIL_BASE : TAIL_BASE + TAIL_PTS, :],
                in_=yt[WGRID * TAIL_K :, :],
            )

            stat = 0
            cnt = 0
            for qb in range(QB):
                ks = pairs[qb]
                ot = op.tile([P, ROWS_PER_CORE], bf16, tag="ot", name="ot")
                for rb in range(NRB):
                    ps = pp.tile([P, RB], f32, tag="ps")
                    for t, k in enumerate(ks):
                        if k == TAIL_K:
                            ksl = slice(TAIL_BASE, TAIL_BASE + TAIL_PTS)
                        else:
                            ksl = slice(0, P)
                        nc.tensor.matmul(
                            ps[:],
                            g_t[ksl, (stat + t) * P : (stat + t + 1) * P],
                            win[k][ksl, rb * RB : (rb + 1) * RB],
                            start=(t == 0),
                            stop=(t == len(ks) - 1),
                        )
                    dst = ot[:, rb * RB : (rb + 1) * RB]
                    # alternate the PSUM->SBUF cast copies between DVE and
                    # ACT copy-by-copy (runs on one engine would serialize
                    # PSUM bank recycling and stall the matmul stream);
                    # DVE takes the odd extra — ACT also issues out-DMAs
                    if cnt % 13 % 2 == 0:
                        nc.vector.tensor_copy(out=dst, in_=ps[:])
                    else:
                        nc.scalar.copy(dst, ps[:])
                    cnt += 1
                stat += len(ks)
                # transposed output strip [128 sorted queries, all rows].
                # Early strips ride the second HWDGE ring (ACT) to overlap
                # the window loads on the sync ring; late strips move to the
                # sync ring (drained by then) to offload the ACT sequencer.
                eng = nc.scalar if qb < QB // 2 else nc.sync
                eng.dma_start(out=out[qb * P : (qb + 1) * P, :], in_=ot[:])

    nc.compile()
    return nc


def _get_plan_and_nc(x_new):
    import ml_dtypes

    key = np.asarray(x_new, dtype=np.float32).tobytes()
    if key not in _PLAN_CACHE:
        pairs, order, idxs, w = _make_plan(x_new)
        si = idxs[order]
        sw = w[order]
        kq = si // WGRID
        n_stat = sum(len(ks) for ks in pairs)
        # G stationaries: [128 pts-in-window, n_stat*128], one 128-col group
        # per (block, window) pair; zero cols for queries of other windows
        gmat = np.zeros((P, n_stat * P), dtype=np.float32)
        stat = 0
        for qb, ks in enumerate(pairs):
            sl = slice(qb * P, (qb + 1) * P)
            for k in ks:
                mask = kq[sl] == k
                cols = stat * P + np.flatnonzero(mask)
                siq = si[sl][mask]
                swq = sw[sl][mask]
                gmat[_win_row(k, siq), cols] = 1.0 - swq
                gmat[_win_row(k, siq + 1), cols] = swq
                stat += 1
        gmat = gmat.astype(ml_dtypes.bfloat16)
        inv = np.argsort(order, kind="stable")  # natural col -> sorted pos
        _PLAN_CACHE[key] = (pairs, n_stat, gmat, inv)
    pairs, n_stat, gmat, inv = _PLAN_CACHE[key]

    nc_key = tuple(tuple(ks) for ks in pairs)
    if nc_key not in _NC_CACHE:
        _NC_CACHE[nc_key] = _build_nc(pairs, n_stat)
    return gmat, inv, _NC_CACHE[nc_key]


def run(y_points, x_new, trace=False, **spmd_kwargs):
    """Run the Bass kernel; returns (output, BassKernelResults)."""
    import ml_dtypes
    from concourse.bass_utils import run_bass_kernel_spmd

    gmat, inv, nc = _get_plan_and_nc(x_new)

    y16 = np.asarray(y_points, dtype=np.float32).astype(ml_dtypes.bfloat16)
    in_maps = []
    for c in range(N_CORES):
        ytc = np.ascontiguousarray(y16[c * ROWS_PER_CORE : (c + 1) * ROWS_PER_CORE].T)
        in_maps.append({"yt": ytc, "g": gmat})

    res = run_bass_kernel_spmd(
        nc, in_maps, list(range(N_CORES)), trace=trace, **spmd_kwargs
    )

    out_full = np.empty((BATCH, M), dtype=np.float32)
    for c in range(N_CORES):
        o = res.results[c]["out"]  # [M sorted queries, rows] bf16
        nat = np.asarray(o).astype(np.float32)[inv, :]  # row-gather to natural
        out_full[c * ROWS_PER_CORE : (c + 1) * ROWS_PER_CORE, :] = nat.T
    return out_full, res


def kernel(y_points, x_new):
    out, _ = run(y_points, x_new)
    return out
